# revision 1
# baseline (speedup 1.0000x reference)
"""GATv2Conv GNN message-passing kernel for 8 Trainium2 NeuronCores.

Strategy (self-contained — hardcoded for the 50000x116 / 800k-edge / 100-graph
problem shape, but parametrized from input shapes):
  * Host: append self-loops, sort edges by destination, shard contiguous graph
    ranges across 8 cores balancing edge counts, lay out per-edge source
    features [x[src]; ea; 1] as a bf16 stream (pure indexing/layout, no math).
  * Device (main SPMD program, per core):
      - xr table = x_T @ [Wr;br] per 128-node window (PE), stored to HBM bf16 (256B rows).
      - per 128-edge chunk: s = x_aug.T @ [Wl;We;bl] (PE) + xr[dst] via
        dma_gather + identity-matmul accumulate (PE); leaky via ACT Prelu
        (alpha honored on HW; sim falls back to relu_mm fold); logits =
        reduce(t*att) (DVE 2x, real-stride att); ex = exp (ACT) expanded to
        64 lanes by ACT copy so msg=gl*ex runs at DVE 2x; one-hot built by
        8x tensor_scalar is_equal (4x mode); scatter via one-hot matmul into
        per-window PSUM. exp/msg/scatter are skewed one group behind so the
        strict-FIFO ACT/DVE/PE queues never head-of-line block on each other.
      - per window: normalize by softmax denominator, accumulate per-graph
        sums of h and h^2 plus x-sums via one-hot matmuls into PSUM.
  * Device (tail SPMD program): global BN stats from per-core partials,
    BN-affine + residual fold, 2-layer MLP head. All cores compute the same
    tiny thing; core 0's output is used.
  * Host: reassemble [100, 2] output (pure indexing).
"""

import os
import numpy as np
import ml_dtypes

os.environ.setdefault("NEURON_RT_RESET_CORES", "1")
bf16 = ml_dtypes.bfloat16

P = 128
HEADS = 4
OUT_C = 16
D = 64
GSLOT = 16
GB_CHUNKS = 8  # chunks per dma_gather batch (ring limit: <2048 idxs)
NEG_SLOPE = 0.2
BN_EPS = 1e-5

_prog_cache = {}


# --------------------------------------------------------------------------
# host prep
# --------------------------------------------------------------------------

def _prep(inputs):
    x = np.asarray(inputs["x"], np.float32)
    ei = np.asarray(inputs["edge_index"], np.int32)
    ea = np.asarray(inputs["edge_attr"], np.float32)
    batch = np.asarray(inputs["batch"], np.int32)
    N, IN_C = x.shape
    E = ei.shape[1]
    G = int(batch.max()) + 1 if batch.size else 1
    G = max(G, 100) if N == 50000 else G  # fixed 100 graphs for this problem
    NC = 8
    CH = IN_C + 2           # x | ea | ones
    CHX = IN_C + 1          # x | ones

    src = np.concatenate([ei[0], np.arange(N, dtype=np.int32)])
    dst = np.concatenate([ei[1], np.arange(N, dtype=np.int32)])
    eav = np.concatenate([ea[:, 0], np.ones(N, np.float32)])
    order = np.argsort(dst, kind="stable")
    ss, ds, es = src[order], dst[order], eav[order]
    ET = ss.shape[0]

    nb = np.searchsorted(batch, np.arange(G + 1))          # node range per graph
    ecnt_g = np.bincount(batch[ds], minlength=G)            # edges per dst-graph
    csum = np.cumsum(ecnt_g)
    gb = [0]
    for k in range(1, NC):
        b = int(np.searchsorted(csum, ET * k / NC))
        gb.append(min(max(b, gb[-1] + 1), G - (NC - k)))
    gb.append(G)
    gb = np.array(gb, np.int64)

    cores = []
    Wmax, CPWmax = 1, 1
    for k in range(NC):
        g0, g1 = int(gb[k]), int(gb[k + 1])
        assert g1 - g0 <= GSLOT, f"core {k} has {g1-g0} graphs > {GSLOT}"
        n0, n1 = int(nb[g0]), int(nb[g1])
        e0, e1 = np.searchsorted(ds, [n0, n1])
        nloc = n1 - n0
        W = max(1, -(-nloc // P))
        rel = ds[e0:e1] - n0
        wofs = np.searchsorted(rel, np.arange(W + 1) * P)
        wcnt = np.diff(wofs)
        CPW = max(1, int(-(-wcnt.max() // P))) if wcnt.size else 1
        Wmax = max(Wmax, W)
        CPWmax = max(CPWmax, CPW)
        cores.append(dict(g0=g0, g1=g1, n0=n0, n1=n1, e0=int(e0), e1=int(e1),
                          rel=rel, wofs=wofs))

    W, CPW = Wmax, CPWmax
    T = W * CPW
    T8 = -(-T // GB_CHUNKS) * GB_CHUNKS
    L = T8 * P

    # shared weight prep
    Wl, bl = np.asarray(inputs["Wl"], np.float32), np.asarray(inputs["bl"], np.float32)
    Wr, br = np.asarray(inputs["Wr"], np.float32), np.asarray(inputs["br"], np.float32)
    We = np.asarray(inputs["We"], np.float32)
    att = np.asarray(inputs["att"], np.float32)
    waug = np.zeros((CH, 2 * D), np.float32)
    waug[:IN_C, :D] = Wl
    waug[:IN_C, D:] = Wl
    waug[IN_C, :D] = We[0]
    waug[CH - 1, :D] = bl
    waug[CH - 1, D:] = bl
    wr = np.concatenate([Wr, br[None, :]], 0)               # [CHX, 64]
    wres = np.concatenate([np.asarray(inputs["Wres"], np.float32),
                           np.asarray(inputs["bres"], np.float32)[None, :]], 0)
    w1 = np.concatenate([np.asarray(inputs["W1"], np.float32),
                         np.asarray(inputs["b1"], np.float32)[None, :]], 0)
    w2 = np.concatenate([np.asarray(inputs["W2"], np.float32),
                         np.asarray(inputs["b2"], np.float32)[None, :]], 0)
    attc = np.tile(att.reshape(1, D), (P, 8))
    iotac = np.tile(np.arange(P, dtype=np.float32), (P, 1))
    identc = np.eye(P, dtype=np.float32)
    nidentc = -np.eye(P, dtype=np.float32)
    misc = np.zeros((D, 8), np.float32)
    misc[:, 0] = np.asarray(inputs["gamma"], np.float32)
    misc[:, 1] = np.asarray(inputs["beta"], np.float32)
    misc[:, 2] = np.asarray(inputs["gat_bias"], np.float32)
    misc[:, 3] = BN_EPS

    cnt_g = (nb[1:] - nb[:-1]).astype(np.float32)

    shared = dict(
        waug=waug.astype(bf16), wr=wr.astype(bf16), wres=wres.astype(bf16),
        attc=attc.astype(bf16), iotac=iotac.astype(bf16),
        identc=identc.astype(bf16), nidentc=nidentc.astype(bf16),
        w1=w1.astype(bf16), w2=w2.astype(bf16), misc=misc,
    )

    in_maps = []
    for k in range(NC):
        c = cores[k]
        n0, n1, e0 = c["n0"], c["n1"], c["e0"]
        nloc = n1 - n0
        relc = c["rel"]
        wofs = c["wofs"]
        Wk = len(wofs) - 1

        sel = np.full(L, -1, np.int64)          # local edge position within core
        for w in range(Wk):
            cnt = wofs[w + 1] - wofs[w]
            if cnt:
                base = w * CPW * P
                sel[base:base + cnt] = wofs[w] + np.arange(cnt)
        valid = sel >= 0
        seli = np.where(valid, sel, 0)
        relv = relc[seli] if relc.size else np.zeros(L, np.int64)

        xga = np.zeros((CH, L), np.float32)
        xga[:IN_C] = np.where(valid, x[ss[e0 + seli]].T, 0.0)
        xga[IN_C] = np.where(valid, es[e0 + seli], 0.0)
        xga[CH - 1] = valid.astype(np.float32)

        pos_w = np.minimum(np.arange(L) // (CPW * P), W - 1)
        dstrel = np.where(valid, relv - pos_w * P, -1.0)
        dstrel = dstrel.astype(np.float32).reshape(T8, P).T    # [128, T8]

        idxv = np.where(valid, relv, 0).astype(np.int16)
        dsti = np.tile(idxv.reshape(-1, 16).T, (8, 1))          # [128, L/16]

        xt = np.zeros((CHX, W * P), np.float32)
        xt[:IN_C, :nloc] = x[n0:n1].T
        xt[IN_C, :nloc] = 1.0

        xnm_a = np.zeros((W * P, CHX), np.float32)
        xnm_a[:nloc, :IN_C] = x[n0:n1]
        xnm_a[:nloc, IN_C] = 1.0
        xnm = xnm_a.reshape(W, P, CHX).transpose(1, 0, 2).reshape(P, W * CHX)

        gm_a = np.zeros((W * P, 2 * GSLOT), np.float32)
        gsl = batch[n0:n1] - c["g0"]
        ar = np.arange(nloc)
        gm_a[ar, gsl] = 1.0
        gm_a[ar, GSLOT + gsl] = 1.0 / np.maximum(cnt_g[c["g0"]:c["g1"]], 1.0)[gsl]
        gmat = gm_a.reshape(W, P, 2 * GSLOT).transpose(1, 0, 2).reshape(P, W * 2 * GSLOT)

        m = dict(
            xga=xga.astype(bf16), dstrel=dstrel, dsti=dsti,
            xt=xt.astype(bf16), xnm=xnm.astype(bf16), gmat=gmat.astype(bf16),
        )
        for kk in ("waug", "wr", "wres", "attc", "iotac", "identc", "nidentc"):
            m[kk] = shared[kk]
        in_maps.append(m)

    meta = dict(N=N, IN_C=IN_C, CH=CH, CHX=CHX, G=G, NC=NC, W=W, CPW=CPW,
                T8=T8, gb=gb, cnt_g=cnt_g)
    return meta, in_maps, shared


# --------------------------------------------------------------------------
# bass programs
# --------------------------------------------------------------------------

def _build_main(meta, leaky_mode="relu_mm", debug=False, dbg_taps=False, ablate=()):
    import concourse.bacc as bacc
    import concourse.mybir as mybir
    import concourse.tile as tile

    F32 = mybir.dt.float32
    BF = mybir.dt.bfloat16
    I16 = mybir.dt.int16
    AL = mybir.AluOpType
    AF = mybir.ActivationFunctionType
    AX = mybir.AxisListType

    CH, CHX, W, CPW, T8 = meta["CH"], meta["CHX"], meta["W"], meta["CPW"], meta["T8"]
    NG = T8 // 8
    NB = T8 // GB_CHUNKS
    GS2 = 2 * GSLOT

    nc = bacc.Bacc(None, target_bir_lowering=False, debug=debug)

    t_xga = nc.dram_tensor("xga", [CH, T8 * P], BF, kind="ExternalInput")
    t_dstrel = nc.dram_tensor("dstrel", [P, T8], F32, kind="ExternalInput")
    t_dsti = nc.dram_tensor("dsti", [P, T8 * P // 16], I16, kind="ExternalInput")
    t_xt = nc.dram_tensor("xt", [CHX, W * P], BF, kind="ExternalInput")
    t_xnm = nc.dram_tensor("xnm", [P, W * CHX], BF, kind="ExternalInput")
    t_gmat = nc.dram_tensor("gmat", [P, W * GS2], BF, kind="ExternalInput")
    t_waug = nc.dram_tensor("waug", [CH, 2 * D], BF, kind="ExternalInput")
    t_wr = nc.dram_tensor("wr", [CHX, D], BF, kind="ExternalInput")
    t_wres = nc.dram_tensor("wres", [CHX, D], BF, kind="ExternalInput")
    t_attc = nc.dram_tensor("attc", [P, 8 * D], BF, kind="ExternalInput")
    t_iotac = nc.dram_tensor("iotac", [P, P], BF, kind="ExternalInput")
    t_id = nc.dram_tensor("identc", [P, P], BF, kind="ExternalInput")
    t_nid = nc.dram_tensor("nidentc", [P, P], BF, kind="ExternalInput")

    o_s = nc.dram_tensor("o_s", [P, 1], F32, kind="ExternalOutput")
    o_hdiv = nc.dram_tensor("o_hdiv", [D, GSLOT], F32, kind="ExternalOutput")
    o_res = nc.dram_tensor("o_res", [D, GSLOT], F32, kind="ExternalOutput")

    xrtab = nc.dram_tensor("xrtab", [W * P, P], BF)
    if dbg_taps:
        d_t = nc.dram_tensor("d_t", [P, 8, D], F32, kind="ExternalOutput")
        d_lg = nc.dram_tensor("d_lg", [P, 8, HEADS], F32, kind="ExternalOutput")
        d_msg = nc.dram_tensor("d_msg", [P, 8, D + HEADS], F32, kind="ExternalOutput")
        d_oh = nc.dram_tensor("d_oh", [P, 8, P], F32, kind="ExternalOutput")
        d_gr = nc.dram_tensor("d_gr", [P, 8, D], F32, kind="ExternalOutput")
        d_win = nc.dram_tensor("d_win", [P, D + HEADS], F32, kind="ExternalOutput")
        d_s = nc.dram_tensor("d_s", [P, 8, D], F32, kind="ExternalOutput")
        d_gl = nc.dram_tensor("d_gl", [P, 8, D], F32, kind="ExternalOutput")

    with tile.TileContext(nc) as tc:
        with tc.tile_pool(name="cst", bufs=1) as cst, \
             tc.tile_pool(name="sgl", bufs=2, space="PSUM") as ps_sgl_pool, \
             tc.tile_pool(name="win", bufs=2, space="PSUM") as ps_win_pool, \
             tc.tile_pool(name="acc", bufs=1, space="PSUM") as ps_acc_pool, \
             tc.tile_pool(name="xsm", bufs=1, space="PSUM") as ps_xsm_pool, \
             tc.tile_pool(name="str", bufs=4) as strm, \
             tc.tile_pool(name="gat", bufs=3) as gatp, \
             tc.tile_pool(name="wrk", bufs=3) as wrk:

            def load_const(t, shape, dtype):
                s = cst.tile(shape, dtype, tag=t.name)
                nc.sync.dma_start(s[:], t[:])
                return s

            # phase-B-critical consts first: HWDGE drains in FIFO order, so
            # xt/wr must not queue behind the 1.9MB dsti load
            xt_t = load_const(t_xt, [CHX, W * P], BF)
            wr_t = load_const(t_wr, [CHX, D], BF)
            xnm_t = load_const(t_xnm, [P, W * CHX], BF)
            gmat_t = load_const(t_gmat, [P, W * GS2], BF)
            waug_t = load_const(t_waug, [CH, 2 * D], BF)
            iotac_t = load_const(t_iotac, [P, P], BF)
            id_t = load_const(t_id, [P, P], BF)
            dstrel_t = load_const(t_dstrel, [P, T8], F32)
            dsti_t = load_const(t_dsti, [P, T8 * P // 16], I16)
            attc_t = load_const(t_attc, [P, 8 * D], BF)
            wres_t = load_const(t_wres, [CHX, D], BF)
            nid_t = load_const(t_nid, [P, P], BF)

            xnm_v = xnm_t[:].rearrange("p (w c) -> p w c", w=W)
            gmat_v = gmat_t[:].rearrange("p (w g) -> p w g", w=W)

            ps_stats = ps_acc_pool.tile([P, GS2], F32, tag="stats")
            ps_xsum = ps_xsm_pool.tile([CHX, GS2], F32, tag="xsum")

            # phase B: xr table (batched 8 windows per psum bank) + x sums
            W8 = -(-W // 8)
            for w8 in range(W8):
                nw = min(8, W - w8 * 8)
                ps_xr = ps_win_pool.tile([P, 8, D], F32, tag="win",
                                         name=f"xr{w8}")
                for j in range(nw):
                    w = w8 * 8 + j
                    nc.tensor.matmul(ps_xr[:, j, :],
                                     xt_t[:, w * P:(w + 1) * P], wr_t[:],
                                     start=(j == 0), stop=True,
                                     skip_group_check=True)
                sb_xr = wrk.tile([P, 8, P], BF, tag="xrw", name=f"xrw{w8}")
                nc.vector.memset(sb_xr[:, :, D:P], 0.0)
                nc.scalar.activation(sb_xr[:, 0:nw, 0:D], ps_xr[:, 0:nw, :],
                                     AF.Copy)
                nc.sync.dma_start(
                    xrtab[w8 * 8 * P:w8 * 8 * P + nw * P, :].rearrange(
                        "(w p) f -> p w f", p=P),
                    sb_xr[:, 0:nw, :])
            for w in range(W):
                nc.tensor.matmul(ps_xsum[:], xnm_v[:, w, :], gmat_v[:, w, :],
                                 start=(w == 0), stop=(w == W - 1),
                                 skip_group_check=True)

            # phase C: edge loop (scatter matmuls skewed one group behind so
            # PE never stalls on the DVE logits chain)
            win_tiles = {}
            gr_tile = None
            pend = []

            def emit_scatter(gq, oh_q, msg_q, gl_q, lg_q):
                sb_exq = wrk.tile([P, 8, D], BF, tag="exq", name=f"exq{gq}")
                nc.scalar.activation(
                    sb_exq[:].rearrange("p c (h k) -> p c h k", k=OUT_C),
                    msg_q[:, :, D:D + HEADS].unsqueeze(3).to_broadcast(
                        [P, 8, HEADS, OUT_C]),
                    AF.Copy)
                nc.vector.tensor_tensor(
                    out=msg_q[:, :, 0:D], in0=gl_q[:], in1=sb_exq[:],
                    op=AL.mult)
                flush = []
                for c8 in range(8):
                    c = gq * 8 + c8
                    w = min(c // CPW, W - 1)
                    if w not in win_tiles:
                        win_tiles[w] = ps_win_pool.tile([P, D + HEADS], F32,
                                                        tag="win", name=f"win{gq}_{w}")
                    first = (c % CPW == 0) and c < W * CPW
                    last = (c == (w + 1) * CPW - 1) if w < W - 1 else (c == T8 - 1)
                    nc.tensor.matmul(win_tiles[w][:], oh_q[:, c8, :],
                                     msg_q[:, c8, :], start=first, stop=last,
                                     skip_group_check=True)
                    if last:
                        flush.append(w)
                return flush

            def do_flush(flush):
                for w in flush:
                    ps_w = win_tiles.pop(w)
                    sb_den = wrk.tile([P, HEADS], F32, tag="den", name=f"den{w}")
                    nc.vector.tensor_scalar(sb_den[:], ps_w[:, D:D + HEADS],
                                            1e-20, None, AL.add)
                    sb_rd = wrk.tile([P, HEADS], F32, tag="rd", name=f"rd{w}")
                    nc.vector.reciprocal(sb_rd[:], sb_den[:])
                    sb_hh2 = wrk.tile([P, 2 * D], BF, tag="hh2", name=f"hh2{w}")
                    nc.vector.tensor_tensor(
                        out=sb_hh2[:, 0:D].rearrange("p (h k) -> p h k", k=OUT_C),
                        in0=ps_w[:, 0:D].rearrange("p (h k) -> p h k", k=OUT_C),
                        in1=sb_rd[:].unsqueeze(2).to_broadcast([P, HEADS, OUT_C]),
                        op=AL.mult)
                    nc.scalar.activation(sb_hh2[:, D:2 * D], sb_hh2[:, 0:D],
                                         AF.Square)
                    nc.tensor.matmul(ps_stats[:], sb_hh2[:], gmat_v[:, w, :],
                                     start=(w == 0), stop=(w == W - 1),
                                     skip_group_check=True)

            for g in range(NG):
                xga_t = strm.tile([CH, 8 * P], BF, tag="xga")
                nc.sync.dma_start(xga_t[:], t_xga[:, g * 8 * P:(g + 1) * 8 * P])
                if g % (GB_CHUNKS // 8) == 0:
                    b = g // (GB_CHUNKS // 8)
                    gr_tile = gatp.tile([P, GB_CHUNKS, P], BF, tag="gr")
                    nidx = GB_CHUNKS * P
                    nc.gpsimd.dma_gather(
                        out_ap=gr_tile[:],
                        in_ap=xrtab[:],
                        idxs_ap=dsti_t[:, b * (nidx // 16):(b + 1) * (nidx // 16)],
                        num_idxs=nidx, num_idxs_reg=nidx, elem_size=P)

                ps_sgl = ps_sgl_pool.tile([P, 8, 2 * D], F32, tag="sgl")
                for c8 in range(8):
                    nc.tensor.matmul(ps_sgl[:, c8, :],
                                     xga_t[:, c8 * P:(c8 + 1) * P], waug_t[:],
                                     start=(c8 % 4 == 0), stop=True,
                                     skip_group_check=True)
                goff = (g % (GB_CHUNKS // 8)) * 8
                if "grmm" not in ablate:
                    for c8 in range(8):
                        nc.tensor.matmul(ps_sgl[:, c8, 0:D], id_t[:],
                                         gr_tile[:, goff + c8, 0:D],
                                         start=False, stop=True, skip_group_check=True)

                if dbg_taps and g == 0:
                    dsf = wrk.tile([P, 8, D], F32, tag="dsf")
                    nc.scalar.activation(dsf[:], ps_sgl[:, :, 0:D], AF.Copy)
                    nc.sync.dma_start(d_s[:], dsf[:])
                    dglf = wrk.tile([P, 8, D], F32, tag="dglf")
                    nc.scalar.activation(dglf[:], ps_sgl[:, :, D:2 * D], AF.Copy)
                    nc.sync.dma_start(d_gl[:], dglf[:])

                sb_t = wrk.tile([P, 8, D], BF, tag="t")
                if leaky_mode == "prelu":
                    nc.scalar.activation(sb_t[:], ps_sgl[:, :, 0:D], AF.Prelu,
                                         alpha=NEG_SLOPE)
                else:
                    sb_r2 = wrk.tile([P, 8, D], BF, tag="r2")
                    nc.scalar.activation(sb_r2[:], ps_sgl[:, :, 0:D], AF.Relu,
                                         scale=-(1.0 - NEG_SLOPE))
                    for c8 in range(8):
                        nc.tensor.matmul(ps_sgl[:, c8, 0:D], id_t[:],
                                         sb_r2[:, c8, :],
                                         start=False, stop=True,
                                         skip_group_check=True)
                    nc.scalar.activation(sb_t[:], ps_sgl[:, :, 0:D], AF.Copy)
                if pend:
                    _, _, pmsg, _, plg = pend[-1]
                    nc.scalar.activation(pmsg[:, :, D:D + HEADS], plg[:], AF.Exp)
                sb_gl = wrk.tile([P, 8, D], BF, tag="gl")
                nc.scalar.activation(sb_gl[:], ps_sgl[:, :, D:2 * D], AF.Copy)

                sb_u = wrk.tile([P, 8, D], BF, tag="u")
                nc.vector.tensor_tensor(
                    out=sb_u[:], in0=sb_t[:],
                    in1=attc_t[:].rearrange("p (c f) -> p c f", c=8),
                    op=AL.mult)
                sb_lg = wrk.tile([P, 8, HEADS], F32, tag="lg")
                nc.vector.tensor_reduce(
                    out=sb_lg[:],
                    in_=sb_u[:].rearrange("p c (h k) -> p c h k", k=OUT_C),
                    axis=AX.X, op=AL.add)
                sb_msg = wrk.tile([P, 8, D + HEADS], BF, tag="msg")

                oh_t = wrk.tile([P, 8, P], BF, tag="oh")
                if "oh" not in ablate:
                    for c8 in range(8):
                        nc.vector.tensor_scalar(
                            oh_t[:, c8, :], iotac_t[:],
                            dstrel_t[:, g * 8 + c8:g * 8 + c8 + 1], None,
                            AL.is_equal)

                if dbg_taps and g == 0:
                    dtf = wrk.tile([P, 8, D], F32, tag="dtf")
                    nc.vector.tensor_copy(dtf[:], sb_t[:])
                    nc.sync.dma_start(d_t[:], dtf[:])
                    nc.sync.dma_start(d_lg[:], sb_lg[:])
                    dmf = wrk.tile([P, 8, D + HEADS], F32, tag="dmf")
                    nc.vector.tensor_copy(dmf[:], sb_msg[:])
                    nc.sync.dma_start(d_msg[:], dmf[:])
                    dof = wrk.tile([P, 8, P], F32, tag="dof")
                    nc.vector.tensor_copy(dof[:], oh_t[:])
                    nc.sync.dma_start(d_oh[:], dof[:])
                    dgf = wrk.tile([P, 8, D], F32, tag="dgf")
                    nc.vector.tensor_copy(dgf[:], gr_tile[:, goff:goff + 8, 0:D])
                    nc.sync.dma_start(d_gr[:], dgf[:])

                pend.append((g, oh_t, sb_msg, sb_gl, sb_lg))
                if len(pend) > 1:
                    do_flush(emit_scatter(*pend.pop(0)))

            while pend:
                _, _, pmsg, _, plg = pend[0]
                nc.scalar.activation(pmsg[:, :, D:D + HEADS], plg[:], AF.Exp)
                do_flush(emit_scatter(*pend.pop(0)))

            # phase D: outputs
            sb_sloc = wrk.tile([P, 1], F32, tag="sloc")
            nc.vector.tensor_reduce(out=sb_sloc[:], in_=ps_stats[:, 0:GSLOT],
                                    axis=AX.X, op=AL.add)
            nc.sync.dma_start(o_s[:], sb_sloc[:])
            sb_hdiv = wrk.tile([D, GSLOT], F32, tag="hdiv")
            nc.scalar.activation(sb_hdiv[:], ps_stats[0:D, GSLOT:GS2], AF.Copy)
            nc.sync.dma_start(o_hdiv[:], sb_hdiv[:])
            sb_xdiv = wrk.tile([CHX, GSLOT], BF, tag="xdiv")
            nc.scalar.activation(sb_xdiv[:], ps_xsum[:, GSLOT:GS2], AF.Copy)
            ps_res = ps_sgl_pool.tile([D, GSLOT], F32, tag="sgl")
            nc.tensor.matmul(ps_res[:], wres_t[:], sb_xdiv[:], start=True,
                             stop=True, skip_group_check=True)
            sb_res = wrk.tile([D, GSLOT], F32, tag="res")
            nc.scalar.activation(sb_res[:], ps_res[:], AF.Copy)
            nc.sync.dma_start(o_res[:], sb_res[:])

    nc.compile()
    return nc


def _build_tail(meta, debug=False):
    import concourse.bacc as bacc
    import concourse.mybir as mybir
    import concourse.tile as tile

    F32 = mybir.dt.float32
    BF = mybir.dt.bfloat16
    AL = mybir.AluOpType
    AF = mybir.ActivationFunctionType
    AX = mybir.AxisListType

    N = meta["N"]
    NC = meta["NC"]
    GALL = NC * GSLOT  # 128

    FPK = 2 * NC + 2 * GALL + 8
    nc = bacc.Bacc(None, target_bir_lowering=False, debug=debug)
    t_fpk = nc.dram_tensor("t_fpk", [D, FPK], F32, kind="ExternalInput")
    t_wpk = nc.dram_tensor("t_wpk", [D + 1, D + 2], BF, kind="ExternalInput")
    t_out = nc.dram_tensor("t_out", [2, GALL], F32, kind="ExternalOutput")

    with tile.TileContext(nc) as tc:
        with tc.tile_pool(name="sb", bufs=1) as sb, \
             tc.tile_pool(name="ps", bufs=2, space="PSUM") as ps:
            fpk = sb.tile([D, FPK], F32, tag="fpk")
            nc.sync.dma_start(fpk[:], t_fpk[:])
            wpk = sb.tile([D + 1, D + 2], BF, tag="wpk")
            nc.sync.dma_start(wpk[:], t_wpk[:])
            s8 = fpk[:, 0:2 * NC]
            hdiv = fpk[:, 2 * NC:2 * NC + GALL]
            res = fpk[:, 2 * NC + GALL:2 * NC + 2 * GALL]
            misc = fpk[:, 2 * NC + 2 * GALL:FPK]
            w1 = wpk[:, 0:D]
            w2 = wpk[:, D:D + 2]

            sh = sb.tile([D, 1], F32, tag="sh")
            nc.vector.tensor_reduce(out=sh[:], in_=s8[:, 0:NC], axis=AX.X, op=AL.add)
            sh2 = sb.tile([D, 1], F32, tag="sh2")
            nc.vector.tensor_reduce(out=sh2[:], in_=s8[:, NC:2 * NC], axis=AX.X,
                                    op=AL.add)
            mu = sb.tile([D, 1], F32, tag="mu")
            nc.scalar.activation(mu[:], sh[:], AF.Copy, scale=1.0 / N)
            e2 = sb.tile([D, 1], F32, tag="e2")
            nc.scalar.activation(e2[:], sh2[:], AF.Copy, scale=1.0 / N)
            mu2 = sb.tile([D, 1], F32, tag="mu2")
            nc.scalar.activation(mu2[:], mu[:], AF.Square)
            var = sb.tile([D, 1], F32, tag="var")
            nc.vector.tensor_tensor(out=var[:], in0=e2[:], in1=mu2[:], op=AL.subtract)
            sd = sb.tile([D, 1], F32, tag="sd")
            nc.scalar.activation(sd[:], var[:], AF.Sqrt, bias=misc[:, 3:4])
            rsd = sb.tile([D, 1], F32, tag="rsd")
            nc.vector.reciprocal(rsd[:], sd[:])
            A = sb.tile([D, 1], F32, tag="A")
            nc.vector.tensor_tensor(out=A[:], in0=misc[:, 0:1], in1=rsd[:], op=AL.mult)
            tmp2 = sb.tile([D, 1], F32, tag="tmp2")
            nc.vector.tensor_tensor(out=tmp2[:], in0=A[:], in1=mu[:], op=AL.mult)
            B = sb.tile([D, 1], F32, tag="B")
            nc.vector.tensor_tensor(out=B[:], in0=misc[:, 1:2], in1=tmp2[:],
                                    op=AL.subtract)

            pooled = sb.tile([D, GALL], F32, tag="pooled")
            nc.vector.tensor_scalar(pooled[:], hdiv, A[:], B[:], AL.mult, AL.add)
            zr = sb.tile([D + 1, GALL], BF, tag="zr")
            nc.vector.memset(zr[D:D + 1, :], 1.0)
            nc.vector.tensor_tensor(out=zr[0:D, :], in0=pooled[:], in1=res,
                                    op=AL.add)
            ps_z = ps.tile([D, GALL], F32, tag="z")
            nc.tensor.matmul(ps_z[:], w1, zr[:], start=True, stop=True)
            z2 = sb.tile([D + 1, GALL], BF, tag="z2")
            nc.vector.memset(z2[D:D + 1, :], 1.0)
            nc.scalar.activation(z2[0:D, :], ps_z[:], AF.Relu)
            ps_o = ps.tile([2, GALL], F32, tag="o")
            nc.tensor.matmul(ps_o[:], w2, z2[:], start=True, stop=True)
            sb_o = sb.tile([2, GALL], F32, tag="out")
            nc.scalar.activation(sb_o[:], ps_o[:], AF.Copy)
            nc.sync.dma_start(t_out[:], sb_o[:])

    nc.compile()
    return nc


# --------------------------------------------------------------------------
# entry point
# --------------------------------------------------------------------------

def _run_sim(nc, in_maps, out_names):
    from concourse.bass_interp import CoreSim
    outs = []
    for m in in_maps:
        sim = CoreSim(nc, require_finite=False, require_nnan=False)
        for name, arr in m.items():
            sim.tensor(name)[:] = arr
        sim.simulate()
        outs.append({n: np.array(sim.tensor(n)) for n in out_names})
    return outs


def kernel(**inputs):
    meta, in_maps, shared = _prep(inputs)
    key = ("main", meta["CH"], meta["W"], meta["CPW"], meta["T8"], _LEAKY_MODE)
    if key not in _prog_cache:
        _prog_cache[key] = _build_main(meta, leaky_mode=_LEAKY_MODE,
                                       debug=(_RUN_MODE == "sim"))
    nc_main = _prog_cache[key]
    tkey = ("tail", meta["N"])
    if tkey not in _prog_cache:
        _prog_cache[tkey] = _build_tail(meta, debug=(_RUN_MODE == "sim"))
    nc_tail = _prog_cache[tkey]

    NC = meta["NC"]
    core_ids = list(range(NC))
    global LAST_EXEC_NS
    if _RUN_MODE == "sim":
        res1 = _run_sim(nc_main, in_maps, ["o_s", "o_hdiv", "o_res"])
        LAST_EXEC_NS = [None]
    else:
        from concourse.bass_utils import run_bass_kernel_spmd
        import time as _time
        _t0 = _time.time()
        r1 = run_bass_kernel_spmd(nc_main, in_maps, core_ids, **_RUN_KW)
        _t1 = _time.time()
        res1 = r1.results
        LAST_EXEC_NS = [getattr(r1, "exec_time_ns", None) or int((_t1 - _t0) * 1e9)]

    s8 = np.zeros((D, 2 * NC), np.float32)
    hdiv = np.zeros((D, NC * GSLOT), np.float32)
    resm = np.zeros((D, NC * GSLOT), np.float32)
    for k in range(NC):
        sk = res1[k]["o_s"]
        s8[:, k] = sk[0:D, 0]
        s8[:, NC + k] = sk[D:2 * D, 0]
        hdiv[:, k * GSLOT:(k + 1) * GSLOT] = res1[k]["o_hdiv"]
        resm[:, k * GSLOT:(k + 1) * GSLOT] = res1[k]["o_res"]

    fpk = np.concatenate([s8, hdiv, resm, shared["misc"]], axis=1).astype(np.float32)
    wpk = np.concatenate([shared["w1"], shared["w2"]], axis=1)
    tail_map = dict(t_fpk=fpk, t_wpk=wpk)
    if _RUN_MODE == "sim":
        res2 = _run_sim(nc_tail, [tail_map], ["t_out"])
        LAST_EXEC_NS.append(None)
    else:
        from concourse.bass_utils import run_bass_kernel_spmd
        import time as _time
        _t0 = _time.time()
        r2 = run_bass_kernel_spmd(nc_tail, [tail_map] * NC, core_ids,
                                  **_RUN_KW_TAIL)
        _t1 = _time.time()
        res2 = r2.results
        LAST_EXEC_NS.append(getattr(r2, "exec_time_ns", None) or int((_t1 - _t0) * 1e9))
    t_out = res2[0]["t_out"]

    G = meta["G"]
    gb = meta["gb"]
    out = np.zeros((G, 2), np.float32)
    for g in range(G):
        k = int(np.searchsorted(gb, g, side="right")) - 1
        slot = g - int(gb[k])
        out[g] = t_out[:, k * GSLOT + slot]
    return out


_LEAKY_MODE = "prelu"
_RUN_MODE = "hw"
_RUN_KW = {}
_RUN_KW_TAIL = {}
LAST_EXEC_NS = None



# revision 2
# speedup vs baseline: 7.3903x; 7.3903x over previous
"""GATv2Conv GNN message-passing kernel for 8 Trainium2 NeuronCores.

Single-launch design optimized for the slow host<->device link:
  * Host ships only compact raw data (~2.6MB/core): the core's node-feature
    shard (transposed bf16), int16 per-edge gather-index streams, bf16 edge
    attrs / relative-dst streams, and tiny weight packs. No host math beyond
    indexing/layout.
  * Device phase A: each core projects its node shard through [Wl|Wr|Wres]
    (one matmul per 128-node window), writes xl rows to a DRAM bounce and xr
    rows to a local DRAM table, and accumulates per-graph xres sums. An
    AllGather publishes the full xl table (messages may source any node);
    xr/xres stay core-local because edges are sharded by destination graph.
  * Device edge loop (per dst window, chunks of 128 edges): dma_gather
    xl[src] from the allgathered table (split in two <32k-row halves to fit
    int16 gather indices; edges are laid out lo-half-first per window) and
    xr[dst] from the local table; s = xl + xr + ea*We via DVE; leaky-relu
    (ACT Prelu); logits = reduce(t*att); exp; msg = xl*exp broadcast; one-hot
    scatter-matmul into per-window PSUM accumulating both the weighted
    message sum and the softmax denominator.
  * Per window: normalize, accumulate per-graph h and h^2 sums via one-hot
    matmul (one-hots built on device from a per-node graph-slot stream).
  * Tail (same launch): AllReduce the [128,1] BN partial sums, finish BN
    affine, add pooled residual, run the 2-layer MLP head per core for its
    own <=16 graphs. Output is [2,16] f32 per core; host reassembles [G,2].
"""

import os
import numpy as np
import ml_dtypes

os.environ.setdefault("NEURON_RT_RESET_CORES", "1")
bf16 = ml_dtypes.bfloat16

P = 128
HEADS = 4
OUT_C = 16
D = 64
GSLOT = 16
NC = 8
NEG_SLOPE = 0.2
BN_EPS = 1e-5

_prog_cache = {}


# --------------------------------------------------------------------------
# host prep (indexing / layout only)
# --------------------------------------------------------------------------

def _pieces(n):
    out = []
    while n > 0:
        m = min(8, n)
        out.append(m)
        n -= m
    return out


def _prep(inputs):
    x = np.asarray(inputs["x"], np.float32)
    ei = np.asarray(inputs["edge_index"], np.int32)
    ea = np.asarray(inputs["edge_attr"], np.float32)
    batch = np.asarray(inputs["batch"], np.int32)
    N, IN_C = x.shape
    CHX = IN_C + 1
    G = int(batch.max()) + 1

    # self loops (edge_attr fill 1.0), sort by destination
    src = np.concatenate([ei[0], np.arange(N, dtype=np.int32)])
    dst = np.concatenate([ei[1], np.arange(N, dtype=np.int32)])
    eav = np.concatenate([ea[:, 0], np.ones(N, np.float32)])
    order = np.argsort(dst, kind="stable")
    ss, ds, es = src[order], dst[order], eav[order]
    ET = ss.shape[0]

    # contiguous graph ranges per core, balanced by edge count
    nb = np.searchsorted(batch, np.arange(G + 1))
    ecnt_g = np.bincount(batch[ds], minlength=G)
    csum = np.cumsum(ecnt_g)
    gb = [0]
    for k in range(1, NC):
        b = int(np.searchsorted(csum, ET * k / NC))
        gb.append(min(max(b, gb[-1] + 1), G - (NC - k)))
    gb.append(G)
    gb = np.array(gb, np.int64)

    cores = []
    Wmax = 1
    for k in range(NC):
        g0, g1 = int(gb[k]), int(gb[k + 1])
        assert g1 - g0 <= GSLOT, f"core {k} has {g1 - g0} graphs > {GSLOT}"
        n0, n1 = int(nb[g0]), int(nb[g1])
        e0, e1 = np.searchsorted(ds, [n0, n1])
        W = max(1, -(-(n1 - n0) // P))
        Wmax = max(Wmax, W)
        cores.append(dict(g0=g0, g1=g1, n0=n0, n1=n1, e0=int(e0), e1=int(e1)))

    WP = Wmax * P
    SPLIT = (NC // 2) * WP

    # padded global node id: core k's nodes live at rows [k*WP, k*WP+nloc)
    pid = np.zeros(N, np.int64)
    for k in range(NC):
        c = cores[k]
        pid[c["n0"]:c["n1"]] = k * WP + np.arange(c["n1"] - c["n0"])

    # per-core edge buckets (window, lo/hi table half), find CPWlo/CPWhi
    CPWlo = 0
    CPWhi = 0
    ebuf = []
    for k in range(NC):
        c = cores[k]
        e0, e1, n0 = c["e0"], c["e1"], c["n0"]
        rel = (ds[e0:e1] - n0).astype(np.int64)
        w_e = rel >> 7
        p_s = pid[ss[e0:e1]]
        hi = p_s >= SPLIT
        ordk = np.lexsort((hi, w_e))
        sk = np.where(hi, p_s - SPLIT, p_s)[ordk]
        relk = rel[ordk]
        ak = es[e0:e1][ordk]
        hik = hi[ordk]
        wk = w_e[ordk]
        nlo = np.bincount(wk[~hik], minlength=Wmax)
        nhi = np.bincount(wk[hik], minlength=Wmax)
        if nlo.max(initial=0):
            CPWlo = max(CPWlo, int(-(-nlo.max() // P)))
        if nhi.max(initial=0):
            CPWhi = max(CPWhi, int(-(-nhi.max() // P)))
        ebuf.append((sk, relk, ak, wk, nlo, nhi))

    CPWlo = max(CPWlo, 1)
    CPWhi = max(CPWhi, 1)
    CPW = CPWlo + CPWhi
    T = Wmax * CPW
    L = T * P

    # shared weight packs
    Wl = np.asarray(inputs["Wl"], np.float32)
    Wr = np.asarray(inputs["Wr"], np.float32)
    Wres = np.asarray(inputs["Wres"], np.float32)
    wpk = np.zeros((CHX, 3 * D), np.float32)
    wpk[:IN_C, 0:D] = Wl
    wpk[:IN_C, D:2 * D] = Wr
    wpk[:IN_C, 2 * D:3 * D] = Wres
    wpk[IN_C, 0:D] = np.asarray(inputs["bl"], np.float32)
    wpk[IN_C, D:2 * D] = np.asarray(inputs["br"], np.float32)
    wpk[IN_C, 2 * D:3 * D] = np.asarray(inputs["bres"], np.float32)

    att = np.asarray(inputs["att"], np.float32)
    We = np.asarray(inputs["We"], np.float32)
    attc = np.tile(att.reshape(1, D), (P, 8))            # [P, 8*D]
    wec = np.tile(We.reshape(1, D), (P, 8))              # [P, 8*D]
    iotac = np.tile(np.arange(P, dtype=np.float32), (P, 1))
    iota16 = np.tile(np.arange(GSLOT, dtype=np.float32), (P, 1))
    misc = np.zeros((D, 4), np.float32)
    misc[:, 0] = np.asarray(inputs["gamma"], np.float32)
    misc[:, 1] = np.asarray(inputs["beta"], np.float32)
    misc[:, 2] = BN_EPS
    w1pk = np.concatenate([np.asarray(inputs["W1"], np.float32),
                           np.asarray(inputs["b1"], np.float32)[None, :]], 0)
    w2pk = np.concatenate([np.asarray(inputs["W2"], np.float32),
                           np.asarray(inputs["b2"], np.float32)[None, :]], 0)

    cnt_g = (nb[1:] - nb[:-1]).astype(np.float32)

    shared = dict(
        t_wpk=wpk.astype(bf16), t_attc=attc.astype(bf16), t_wec=wec.astype(bf16),
        t_iotac=iotac.astype(bf16), t_iota16=iota16.astype(bf16),
        t_misc=misc, t_w1=w1pk.astype(bf16), t_w2=w2pk.astype(bf16),
    )

    starts_lo = (np.arange(Wmax) * CPW) * P
    starts_hi = (np.arange(Wmax) * CPW + CPWlo) * P

    in_maps = []
    for k in range(NC):
        c = cores[k]
        n0, n1 = c["n0"], c["n1"]
        nloc = n1 - n0
        sk, relk, ak, wk, nlo, nhi = ebuf[k]

        sizes = np.stack([nlo, nhi], 1).ravel()
        bstart = np.concatenate([[0], np.cumsum(sizes)[:-1]])
        bases = np.stack([starts_lo, starts_hi], 1).ravel()
        j = np.arange(sk.shape[0])
        bid = np.repeat(np.arange(2 * Wmax), sizes)
        slot = bases[bid] + (j - bstart[bid])

        srci = np.zeros(L, np.int16)
        dsti = np.zeros(L, np.int16)
        eas = np.zeros(L, np.float32)
        drel = np.full(L, -1.0, np.float32)
        srci[slot] = sk.astype(np.int16)
        dsti[slot] = relk.astype(np.int16)
        eas[slot] = ak
        drel[slot] = (relk - wk * P).astype(np.float32)

        xsh = np.zeros((CHX, WP), np.float32)
        xsh[:IN_C, :nloc] = x[n0:n1].T
        xsh[IN_C, :nloc] = 1.0

        gsl = np.full(WP, -1.0, np.float32)
        gsl[:nloc] = (batch[n0:n1] - c["g0"]).astype(np.float32)

        icnt = np.ones(GSLOT, np.float32)
        ng = c["g1"] - c["g0"]
        icnt[:ng] = 1.0 / np.maximum(cnt_g[c["g0"]:c["g1"]], 1.0)

        m = dict(
            t_xsh=xsh.astype(bf16),
            t_srci=srci.reshape(-1, 16).T.copy(),
            t_dsti=dsti.reshape(-1, 16).T.copy(),
            t_ea=eas.reshape(T, P).T.astype(bf16),
            t_drel=drel.reshape(T, P).T.astype(bf16),
            t_gslot=gsl.reshape(Wmax, P).T.astype(bf16),
            t_icnt=np.tile(icnt.reshape(1, GSLOT), (D, 1)),
        )
        m.update(shared)
        in_maps.append(m)

    meta = dict(N=N, IN_C=IN_C, CHX=CHX, G=G, Wmax=Wmax, WP=WP,
                CPWlo=CPWlo, CPWhi=CPWhi, CPW=CPW, T=T, L=L, gb=gb)
    return meta, in_maps


# --------------------------------------------------------------------------
# bass program (single launch, collectives inside)
# --------------------------------------------------------------------------

def _build(meta, leaky_mode="prelu", debug=False):
    import concourse.bacc as bacc
    import concourse.mybir as mybir
    import concourse.tile as tile

    F32 = mybir.dt.float32
    BF = mybir.dt.bfloat16
    I16 = mybir.dt.int16
    AL = mybir.AluOpType
    AF = mybir.ActivationFunctionType
    AX = mybir.AxisListType

    N = meta["N"]
    CHX = meta["CHX"]
    Wmax, WP = meta["Wmax"], meta["WP"]
    CPWlo, CPWhi, CPW = meta["CPWlo"], meta["CPWhi"], meta["CPW"]
    T, L = meta["T"], meta["L"]
    SPLIT = (NC // 2) * WP

    nc = bacc.Bacc(None, target_bir_lowering=False, num_devices=NC, debug=debug)

    t_xsh = nc.dram_tensor("t_xsh", [CHX, WP], BF, kind="ExternalInput")
    t_srci = nc.dram_tensor("t_srci", [16, L // 16], I16, kind="ExternalInput")
    t_dsti = nc.dram_tensor("t_dsti", [16, L // 16], I16, kind="ExternalInput")
    t_ea = nc.dram_tensor("t_ea", [P, T], BF, kind="ExternalInput")
    t_drel = nc.dram_tensor("t_drel", [P, T], BF, kind="ExternalInput")
    t_gslot = nc.dram_tensor("t_gslot", [P, Wmax], BF, kind="ExternalInput")
    t_icnt = nc.dram_tensor("t_icnt", [D, GSLOT], F32, kind="ExternalInput")
    t_wpk = nc.dram_tensor("t_wpk", [CHX, 3 * D], BF, kind="ExternalInput")
    t_attc = nc.dram_tensor("t_attc", [P, 8 * D], BF, kind="ExternalInput")
    t_wec = nc.dram_tensor("t_wec", [P, 8 * D], BF, kind="ExternalInput")
    t_iotac = nc.dram_tensor("t_iotac", [P, P], BF, kind="ExternalInput")
    t_iota16 = nc.dram_tensor("t_iota16", [P, GSLOT], BF, kind="ExternalInput")
    t_misc = nc.dram_tensor("t_misc", [D, 4], F32, kind="ExternalInput")
    t_w1 = nc.dram_tensor("t_w1", [D + 1, D], BF, kind="ExternalInput")
    t_w2 = nc.dram_tensor("t_w2", [D + 1, 2], BF, kind="ExternalInput")

    o_out = nc.dram_tensor("o_out", [2, GSLOT], F32, kind="ExternalOutput")

    xl_sh = nc.dram_tensor("xl_sh", [WP, D], F32)
    xl_full = nc.dram_tensor("xl_full", [NC * WP, D], F32)
    xr_tab = nc.dram_tensor("xr_tab", [WP, D], F32)
    bn_in = nc.dram_tensor("bn_in", [P, 1], F32)
    bn_out = nc.dram_tensor("bn_out", [P, 1], F32)

    PL = _pieces(CPWlo)
    PH = _pieces(CPWhi)

    with tile.TileContext(nc) as tc:
        with tc.tile_pool(name="cst", bufs=1) as cst, \
             tc.tile_pool(name="pa", bufs=2, space="PSUM") as pa_pool, \
             tc.tile_pool(name="win", bufs=2, space="PSUM") as win_pool, \
             tc.tile_pool(name="acc", bufs=1, space="PSUM") as acc_pool, \
             tc.tile_pool(name="rsm", bufs=1, space="PSUM") as rsm_pool, \
             tc.tile_pool(name="gat", bufs=3) as gatp, \
             tc.tile_pool(name="wrk", bufs=3) as wrk:

            def load_const(t, shape, dtype):
                s = cst.tile(shape, dtype, tag=t.name)
                nc.sync.dma_start(s[:], t[:])
                return s

            # phase-A-critical consts first (HWDGE drains FIFO)
            xsh_t = load_const(t_xsh, [CHX, WP], BF)
            wpk_t = load_const(t_wpk, [CHX, 3 * D], BF)
            gslot_t = load_const(t_gslot, [P, Wmax], BF)
            iota16_t = load_const(t_iota16, [P, GSLOT], BF)
            iotac_t = load_const(t_iotac, [P, P], BF)
            attc_t = load_const(t_attc, [P, 8 * D], BF)
            wec_t = load_const(t_wec, [P, 8 * D], BF)
            misc_t = load_const(t_misc, [D, 4], F32)
            icnt_t = load_const(t_icnt, [D, GSLOT], F32)
            w1_t = load_const(t_w1, [D + 1, D], BF)
            w2_t = load_const(t_w2, [D + 1, 2], BF)
            ea_t = load_const(t_ea, [P, T], BF)
            dreb_t = load_const(t_drel, [P, T], BF)
            # gather index streams: replicate [16, L/16] across the 8
            # gpsimd stripes on device
            srct = cst.tile([P, L // 16], I16, tag="srct")
            dstt = cst.tile([P, L // 16], I16, tag="dstt")
            for r in range(8):
                nc.sync.dma_start(srct[16 * r:16 * (r + 1), :], t_srci[:])
                nc.sync.dma_start(dstt[16 * r:16 * (r + 1), :], t_dsti[:])

            drel_t = cst.tile([P, T], F32, tag="drelf")
            nc.vector.tensor_copy(drel_t[:], dreb_t[:])
            gslf_t = cst.tile([P, Wmax], F32, tag="gslf")
            nc.vector.tensor_copy(gslf_t[:], gslot_t[:])
            gm_all = cst.tile([P, Wmax, GSLOT], BF, tag="gmall")

            ps_rsum = rsm_pool.tile([D, GSLOT], F32, tag="rsum")
            ps_stats = acc_pool.tile([P, GSLOT], F32, tag="stats")

            # ---------------- phase A: projection tables -----------------
            for w in range(Wmax):
                ps_a = pa_pool.tile([P, 3 * D], F32, tag="pa")
                nc.tensor.matmul(ps_a[:], xsh_t[:, w * P:(w + 1) * P],
                                 wpk_t[:], start=True, stop=True,
                                 skip_group_check=True)
                sxl = wrk.tile([P, D], F32, tag="sxl")
                nc.scalar.activation(sxl[:], ps_a[:, 0:D], AF.Copy)
                nc.gpsimd.dma_start(xl_sh[w * P:(w + 1) * P, :], sxl[:])
                sxr = wrk.tile([P, D], F32, tag="sxr")
                nc.scalar.activation(sxr[:], ps_a[:, D:2 * D], AF.Copy)
                nc.gpsimd.dma_start(xr_tab[w * P:(w + 1) * P, :], sxr[:])
                sxe = wrk.tile([P, D], BF, tag="sxe")
                nc.scalar.activation(sxe[:], ps_a[:, 2 * D:3 * D], AF.Copy)
                nc.vector.tensor_scalar(gm_all[:, w, :], iota16_t[:],
                                        gslf_t[:, w:w + 1], None, AL.is_equal)
                nc.tensor.matmul(ps_rsum[:], sxe[:], gm_all[:, w, :],
                                 start=(w == 0), stop=(w == Wmax - 1),
                                 skip_group_check=True)

            nc.gpsimd.collective_compute(
                "AllGather", AL.bypass,
                replica_groups=[list(range(NC))],
                ins=[xl_sh[:]], outs=[xl_full[:]],
            )

            # ---------------- edge loop --------------------------------
            for w in range(Wmax):
                win_ps = win_pool.tile([P, D + HEADS], F32, tag="win")
                cw = 0  # chunk index within window
                for run_off, run_pieces, lo in ((0, PL, True),
                                                (CPWlo, PH, False)):
                    po = 0
                    for m in run_pieces:
                        c0 = w * CPW + run_off + po   # global chunk
                        so = c0 * P                   # global slot
                        gx = gatp.tile([P, m, D], F32, tag=f"gx{m}")
                        src_tab = xl_full[0:SPLIT, :] if lo \
                            else xl_full[SPLIT:2 * SPLIT, :]
                        nc.gpsimd.dma_gather(
                            out_ap=gx[:], in_ap=src_tab,
                            idxs_ap=srct[:, so // 16:(so + m * P) // 16],
                            num_idxs=m * P, num_idxs_reg=m * P, elem_size=D)
                        gr = gatp.tile([P, m, D], F32, tag=f"gr{m}")
                        nc.gpsimd.dma_gather(
                            out_ap=gr[:], in_ap=xr_tab[:],
                            idxs_ap=dstt[:, so // 16:(so + m * P) // 16],
                            num_idxs=m * P, num_idxs_reg=m * P, elem_size=D)

                        em = wrk.tile([P, m, D], F32, tag=f"em{m}")
                        nc.vector.tensor_tensor(
                            out=em[:],
                            in0=ea_t[:, c0:c0 + m].unsqueeze(2)
                                .to_broadcast([P, m, D]),
                            in1=wec_t[:, 0:m * D].rearrange(
                                "p (c f) -> p c f", c=m),
                            op=AL.mult)
                        sa = wrk.tile([P, m, D], F32, tag=f"sa{m}")
                        nc.vector.tensor_tensor(out=sa[:], in0=gx[:],
                                                in1=gr[:], op=AL.add)
                        nc.vector.tensor_tensor(out=sa[:], in0=sa[:],
                                                in1=em[:], op=AL.add)
                        sb_t = wrk.tile([P, m, D], BF, tag=f"t{m}")
                        if leaky_mode == "prelu":
                            nc.scalar.activation(sb_t[:], sa[:], AF.Prelu,
                                                 alpha=NEG_SLOPE)
                        else:
                            sb_r = wrk.tile([P, m, D], F32, tag=f"r{m}")
                            nc.scalar.activation(sb_r[:], sa[:], AF.Relu,
                                                 scale=-(1.0 - NEG_SLOPE))
                            nc.vector.tensor_tensor(out=sb_t[:], in0=sa[:],
                                                    in1=sb_r[:], op=AL.add)
                        sb_u = wrk.tile([P, m, D], BF, tag=f"u{m}")
                        nc.vector.tensor_tensor(
                            out=sb_u[:], in0=sb_t[:],
                            in1=attc_t[:, 0:m * D].rearrange(
                                "p (c f) -> p c f", c=m),
                            op=AL.mult)
                        sb_lg = wrk.tile([P, m, HEADS], F32, tag=f"lg{m}")
                        nc.vector.tensor_reduce(
                            out=sb_lg[:],
                            in_=sb_u[:].rearrange("p c (h k) -> p c h k",
                                                  k=OUT_C),
                            axis=AX.X, op=AL.add)
                        exf = wrk.tile([P, m, HEADS], F32, tag=f"ex{m}")
                        nc.scalar.activation(exf[:], sb_lg[:], AF.Exp)
                        exb = wrk.tile([P, m, D], F32, tag=f"exb{m}")
                        nc.scalar.activation(
                            exb[:].rearrange("p c (h k) -> p c h k", k=OUT_C),
                            exf[:].unsqueeze(3).to_broadcast(
                                [P, m, HEADS, OUT_C]),
                            AF.Copy)
                        msg = wrk.tile([P, m, D + HEADS], BF, tag=f"msg{m}")
                        nc.vector.tensor_tensor(out=msg[:, :, 0:D], in0=gx[:],
                                                in1=exb[:], op=AL.mult)
                        nc.scalar.activation(msg[:, :, D:D + HEADS], exf[:],
                                             AF.Copy)
                        oh = wrk.tile([P, m, P], BF, tag=f"oh{m}")
                        for j in range(m):
                            nc.vector.tensor_scalar(
                                oh[:, j, :], iotac_t[:],
                                drel_t[:, c0 + j:c0 + j + 1], None,
                                AL.is_equal)
                        for j in range(m):
                            nc.tensor.matmul(win_ps[:], oh[:, j, :],
                                             msg[:, j, :],
                                             start=(cw + j == 0),
                                             stop=(cw + j == CPW - 1),
                                             skip_group_check=True)
                        po += m
                        cw += m

                # window flush: softmax-normalize, accumulate BN/pool stats
                den = wrk.tile([P, HEADS], F32, tag="den")
                nc.vector.tensor_scalar(den[:], win_ps[:, D:D + HEADS],
                                        1e-20, None, AL.add)
                rd = wrk.tile([P, HEADS], F32, tag="rd")
                nc.vector.reciprocal(rd[:], den[:])
                hh2 = wrk.tile([P, 2 * D], BF, tag="hh2")
                nc.vector.tensor_tensor(
                    out=hh2[:, 0:D].rearrange("p (h k) -> p h k", k=OUT_C),
                    in0=win_ps[:, 0:D].rearrange("p (h k) -> p h k", k=OUT_C),
                    in1=rd[:].unsqueeze(2).to_broadcast([P, HEADS, OUT_C]),
                    op=AL.mult)
                nc.scalar.activation(hh2[:, D:2 * D], hh2[:, 0:D], AF.Square)
                nc.tensor.matmul(ps_stats[:], hh2[:], gm_all[:, w, :],
                                 start=(w == 0), stop=(w == Wmax - 1),
                                 skip_group_check=True)

            # ---------------- BN allreduce + tail ----------------------
            sl = wrk.tile([P, 1], F32, tag="sl")
            nc.vector.tensor_reduce(out=sl[:], in_=ps_stats[:],
                                    axis=AX.X, op=AL.add)
            nc.gpsimd.dma_start(bn_in[:], sl[:])
            nc.gpsimd.collective_compute(
                "AllReduce", AL.add,
                replica_groups=[list(range(NC))],
                ins=[bn_in[:]], outs=[bn_out[:]],
            )
            sh = wrk.tile([D, 2], F32, tag="sh")
            nc.gpsimd.dma_start(sh[:, 0:1], bn_out[0:D, :])
            nc.gpsimd.dma_start(sh[:, 1:2], bn_out[D:2 * D, :])

            mu = wrk.tile([D, 1], F32, tag="mu")
            nc.scalar.activation(mu[:], sh[:, 0:1], AF.Copy, scale=1.0 / N)
            e2 = wrk.tile([D, 1], F32, tag="e2")
            nc.scalar.activation(e2[:], sh[:, 1:2], AF.Copy, scale=1.0 / N)
            mu2 = wrk.tile([D, 1], F32, tag="mu2")
            nc.scalar.activation(mu2[:], mu[:], AF.Square)
            var = wrk.tile([D, 1], F32, tag="var")
            nc.vector.tensor_tensor(out=var[:], in0=e2[:], in1=mu2[:],
                                    op=AL.subtract)
            sd = wrk.tile([D, 1], F32, tag="sd")
            nc.scalar.activation(sd[:], var[:], AF.Sqrt, bias=misc_t[:, 2:3])
            rsd = wrk.tile([D, 1], F32, tag="rsd")
            nc.vector.reciprocal(rsd[:], sd[:])
            cA = wrk.tile([D, 1], F32, tag="cA")
            nc.vector.tensor_tensor(out=cA[:], in0=misc_t[:, 0:1], in1=rsd[:],
                                    op=AL.mult)
            tmp = wrk.tile([D, 1], F32, tag="tmp")
            nc.vector.tensor_tensor(out=tmp[:], in0=cA[:], in1=mu[:],
                                    op=AL.mult)
            cB = wrk.tile([D, 1], F32, tag="cB")
            nc.vector.tensor_tensor(out=cB[:], in0=misc_t[:, 1:2], in1=tmp[:],
                                    op=AL.subtract)

            ph = wrk.tile([D, GSLOT], F32, tag="ph")
            nc.vector.tensor_tensor(out=ph[:], in0=ps_stats[0:D, :],
                                    in1=icnt_t[:], op=AL.mult)
            pooled = wrk.tile([D, GSLOT], F32, tag="pooled")
            nc.vector.tensor_scalar(pooled[:], ph[:], cA[:], cB[:],
                                    AL.mult, AL.add)
            pr = wrk.tile([D, GSLOT], F32, tag="pr")
            nc.vector.tensor_tensor(out=pr[:], in0=ps_rsum[:], in1=icnt_t[:],
                                    op=AL.mult)
            zr = wrk.tile([D + 1, GSLOT], BF, tag="zr")
            nc.vector.memset(zr[D:D + 1, :], 1.0)
            nc.vector.tensor_tensor(out=zr[0:D, :], in0=pooled[:], in1=pr[:],
                                    op=AL.add)
            ps_z = win_pool.tile([D, GSLOT], F32, tag="win")
            nc.tensor.matmul(ps_z[:], w1_t[:], zr[:], start=True, stop=True,
                             skip_group_check=True)
            z2 = wrk.tile([D + 1, GSLOT], BF, tag="z2")
            nc.vector.memset(z2[D:D + 1, :], 1.0)
            nc.scalar.activation(z2[0:D, :], ps_z[:], AF.Relu)
            ps_o = win_pool.tile([2, GSLOT], F32, tag="win")
            nc.tensor.matmul(ps_o[:], w2_t[:], z2[:], start=True, stop=True,
                             skip_group_check=True)
            so = wrk.tile([2, GSLOT], F32, tag="so")
            nc.scalar.activation(so[:], ps_o[:], AF.Copy)
            nc.sync.dma_start(o_out[:], so[:])

    nc.compile()
    return nc


# --------------------------------------------------------------------------
# entry point
# --------------------------------------------------------------------------

def _run_sim(nc, in_maps):
    from concourse.bass_interp import MultiCoreSim
    ms = MultiCoreSim(nc, num_cores=NC, num_workers=NC,
                      require_finite=False, require_nnan=False)
    for k in range(NC):
        for name, arr in in_maps[k].items():
            ms.cores[k].tensor(name)[:] = arr
    ms.simulate()
    return [{"o_out": np.array(ms.cores[k].tensor("o_out"))} for k in range(NC)]


def kernel(**inputs):
    meta, in_maps = _prep(inputs)
    key = ("v2", meta["IN_C"], meta["Wmax"], meta["CPWlo"], meta["CPWhi"],
           meta["N"], _LEAKY_MODE)
    if key not in _prog_cache:
        _prog_cache[key] = _build(meta, leaky_mode=_LEAKY_MODE,
                                  debug=(_RUN_MODE == "sim"))
    nc = _prog_cache[key]

    global LAST_EXEC_NS
    if _RUN_MODE == "sim":
        res = _run_sim(nc, in_maps)
        LAST_EXEC_NS = [None]
    else:
        from concourse.bass_utils import run_bass_kernel_spmd
        import time as _time
        _t0 = _time.time()
        r = run_bass_kernel_spmd(nc, in_maps, list(range(NC)))
        _t1 = _time.time()
        res = r.results
        LAST_EXEC_NS = [getattr(r, "exec_time_ns", None)
                        or int((_t1 - _t0) * 1e9)]

    G = meta["G"]
    gb = meta["gb"]
    out = np.zeros((G, 2), np.float32)
    for g in range(G):
        k = int(np.searchsorted(gb, g, side="right")) - 1
        slot = g - int(gb[k])
        out[g] = res[k]["o_out"][:, slot]
    return out


_LEAKY_MODE = "prelu"
_RUN_MODE = "hw"
LAST_EXEC_NS = None


# revision 6
# speedup vs baseline: 8.9666x; 1.2133x over previous
"""GATv2Conv GNN message-passing kernel for 8 Trainium2 NeuronCores.

Single-launch design optimized for the slow host<->device link:
  * Host ships only compact raw data (~2.5MB/core) packed into 4 tensors:
    the core's node-feature shard + weight pack (bf16), int16 per-edge
    gather-index streams, a bf16 misc pack (edge attrs, small consts), and a
    tiny f32 pack. No host math beyond indexing/layout.
  * Device phase A: each core projects its node shard through [Wl|Wr|Wres]
    (one matmul per 128-node window), writes xl rows to a DRAM bounce and xr
    rows to a local DRAM table, and accumulates per-graph xres sums. An
    AllGather publishes the full xl table (messages may source any node);
    xr/xres stay core-local because edges are sharded by destination graph.
  * Device edge loop (per dst window, pieces of <=15 128-edge chunks):
    dma_gather xl[src] from the allgathered table (split in two <32k-row
    halves to fit int16 gather indices; edges laid out lo-half-first per
    window), xr[dst] from the local table, and the scatter one-hot rows from
    a device-built identity table; s = xl + xr + ea*We via DVE; leaky-relu
    (ACT Prelu); logits = reduce(t*att); exp; msg = xl*exp; one-hot
    scatter-matmul into per-window PSUM accumulating the weighted message
    sum and the softmax denominator.
  * Per window: normalize, accumulate per-graph h and h^2 sums via one-hot
    matmul (one-hots built on device from a per-node graph-slot stream).
  * Tail (same launch): AllReduce the [128,1] BN partial sums, finish BN
    affine, add pooled residual, run the 2-layer MLP head per core for its
    own <=16 graphs. Output is [2,16] f32 per core; host reassembles [G,2].
"""

import os
import numpy as np
import ml_dtypes

os.environ.setdefault("NEURON_RT_RESET_CORES", "1")
os.environ.setdefault("CONCOURSE_SCRUB_NEFF_DEBUG_INFO", "1")
bf16 = ml_dtypes.bfloat16

P = 128
HEADS = 4
OUT_C = 16
D = 64
GSLOT = 16
NC = 8
NEG_SLOPE = 0.2
BN_EPS = 1e-5
PIECE = 8  # max 128-edge chunks per dma_gather (1024-entry index ring)

_prog_cache = {}


# --------------------------------------------------------------------------
# host prep (indexing / layout only)
# --------------------------------------------------------------------------

def _pieces(n):
    out = []
    while n > 0:
        m = min(PIECE, n)
        out.append(m)
        n -= m
    return out


def _prep(inputs):
    x = np.asarray(inputs["x"], np.float32)
    ei = np.asarray(inputs["edge_index"], np.int32)
    ea = np.asarray(inputs["edge_attr"], np.float32)
    batch = np.asarray(inputs["batch"], np.int32)
    N, IN_C = x.shape
    CHX = IN_C + 1
    G = int(batch.max()) + 1

    # self loops (edge_attr fill 1.0), sort by destination
    src = np.concatenate([ei[0], np.arange(N, dtype=np.int32)])
    dst = np.concatenate([ei[1], np.arange(N, dtype=np.int32)])
    eav = np.concatenate([ea[:, 0], np.ones(N, np.float32)])
    order = np.argsort(dst, kind="stable")
    ss, ds, es = src[order], dst[order], eav[order]
    ET = ss.shape[0]

    # contiguous graph ranges per core, balanced by edge count
    nb = np.searchsorted(batch, np.arange(G + 1))
    ecnt_g = np.bincount(batch[ds], minlength=G)
    csum = np.cumsum(ecnt_g)
    gb = [0]
    for k in range(1, NC):
        b = int(np.searchsorted(csum, ET * k / NC))
        gb.append(min(max(b, gb[-1] + 1), G - (NC - k)))
    gb.append(G)
    gb = np.array(gb, np.int64)

    cores = []
    Wmax = 1
    for k in range(NC):
        g0, g1 = int(gb[k]), int(gb[k + 1])
        assert g1 - g0 <= GSLOT, f"core {k} has {g1 - g0} graphs > {GSLOT}"
        n0, n1 = int(nb[g0]), int(nb[g1])
        e0, e1 = np.searchsorted(ds, [n0, n1])
        W = max(1, -(-(n1 - n0) // P))
        Wmax = max(Wmax, W)
        cores.append(dict(g0=g0, g1=g1, n0=n0, n1=n1, e0=int(e0), e1=int(e1)))

    WP = Wmax * P
    SPLIT = (NC // 2) * WP

    # padded global node id: core k's nodes live at rows [k*WP, k*WP+nloc)
    pid = np.zeros(N, np.int64)
    for k in range(NC):
        c = cores[k]
        pid[c["n0"]:c["n1"]] = k * WP + np.arange(c["n1"] - c["n0"])

    # per-core edge buckets (window, lo/hi table half), find CPWlo/CPWhi
    CPWlo = 0
    CPWhi = 0
    ebuf = []
    for k in range(NC):
        c = cores[k]
        e0, e1, n0 = c["e0"], c["e1"], c["n0"]
        rel = (ds[e0:e1] - n0).astype(np.int64)
        w_e = rel >> 7
        p_s = pid[ss[e0:e1]]
        hi = p_s >= SPLIT
        ordk = np.lexsort((hi, w_e))
        sk = np.where(hi, p_s - SPLIT, p_s)[ordk]
        relk = rel[ordk]
        ak = es[e0:e1][ordk]
        hik = hi[ordk]
        wk = w_e[ordk]
        nlo = np.bincount(wk[~hik], minlength=Wmax)
        nhi = np.bincount(wk[hik], minlength=Wmax)
        if nlo.max(initial=0):
            CPWlo = max(CPWlo, int(-(-nlo.max() // P)))
        if nhi.max(initial=0):
            CPWhi = max(CPWhi, int(-(-nhi.max() // P)))
        ebuf.append((sk, relk, ak, wk, nlo, nhi))

    CPWlo = max(CPWlo, 1)
    CPWhi = max(CPWhi, 1)
    CPW = CPWlo + CPWhi
    T = Wmax * CPW
    L = T * P

    # shared weight packs
    Wl = np.asarray(inputs["Wl"], np.float32)
    Wr = np.asarray(inputs["Wr"], np.float32)
    Wres = np.asarray(inputs["Wres"], np.float32)
    wpk = np.zeros((CHX, 3 * D), np.float32)
    wpk[:IN_C, 0:D] = Wl
    wpk[:IN_C, D:2 * D] = Wr
    wpk[:IN_C, 2 * D:3 * D] = Wres
    wpk[IN_C, 0:D] = np.asarray(inputs["bl"], np.float32)
    wpk[IN_C, D:2 * D] = np.asarray(inputs["br"], np.float32)
    wpk[IN_C, 2 * D:3 * D] = np.asarray(inputs["bres"], np.float32)

    att = np.asarray(inputs["att"], np.float32)
    We = np.asarray(inputs["We"], np.float32)

    # bf16 misc pack [P, ...]: ea | attc | wec | iotac | iota16 | iotap |
    # gslot | w1w2 (rows 0:65)
    o_ea = 0
    o_att = o_ea + T
    o_wec = o_att + PIECE * D
    o_ioc = o_wec + PIECE * D
    o_i16 = o_ioc + P
    o_iop = o_i16 + GSLOT
    o_gsl = o_iop + 1
    o_w12 = o_gsl + Wmax
    BCOLS = o_w12 + D + 2
    bpk = np.zeros((P, BCOLS), np.float32)
    bpk[:, o_att:o_att + PIECE * D] = np.tile(att.reshape(1, D), (P, PIECE))
    bpk[:, o_wec:o_wec + PIECE * D] = np.tile(We.reshape(1, D), (P, PIECE))
    bpk[:, o_ioc:o_ioc + P] = np.tile(np.arange(P, dtype=np.float32), (P, 1))
    bpk[:, o_i16:o_i16 + GSLOT] = np.tile(np.arange(GSLOT, dtype=np.float32),
                                          (P, 1))
    bpk[:, o_iop] = np.arange(P, dtype=np.float32)
    bpk[0:D, o_w12:o_w12 + D] = np.asarray(inputs["W1"], np.float32)
    bpk[D, o_w12:o_w12 + D] = np.asarray(inputs["b1"], np.float32)
    bpk[0:D, o_w12 + D:o_w12 + D + 2] = np.asarray(inputs["W2"], np.float32)
    bpk[D, o_w12 + D:o_w12 + D + 2] = np.asarray(inputs["b2"], np.float32)

    fpk = np.zeros((D, GSLOT + 4), np.float32)
    fpk[:, GSLOT] = np.asarray(inputs["gamma"], np.float32)
    fpk[:, GSLOT + 1] = np.asarray(inputs["beta"], np.float32)
    fpk[:, GSLOT + 2] = BN_EPS

    cnt_g = (nb[1:] - nb[:-1]).astype(np.float32)

    starts_lo = (np.arange(Wmax) * CPW) * P
    starts_hi = (np.arange(Wmax) * CPW + CPWlo) * P

    in_maps = []
    for k in range(NC):
        c = cores[k]
        n0, n1 = c["n0"], c["n1"]
        nloc = n1 - n0
        sk, relk, ak, wk, nlo, nhi = ebuf[k]

        sizes = np.stack([nlo, nhi], 1).ravel()
        bstart = np.concatenate([[0], np.cumsum(sizes)[:-1]])
        bases = np.stack([starts_lo, starts_hi], 1).ravel()
        j = np.arange(sk.shape[0])
        bid = np.repeat(np.arange(2 * Wmax), sizes)
        slot = bases[bid] + (j - bstart[bid])

        i16 = np.zeros((3, L), np.int16)
        i16[0, slot] = sk.astype(np.int16)            # src (table-half local)
        i16[1, slot] = relk.astype(np.int16)          # dst local
        i16[2, :] = P                                  # pad -> zero one-hot row
        i16[2, slot] = (relk & (P - 1)).astype(np.int16)

        xsw = np.zeros((CHX, WP + 3 * D), np.float32)
        xsw[:IN_C, :nloc] = x[n0:n1].T
        xsw[IN_C, :nloc] = 1.0
        xsw[:, WP:] = wpk

        bpkc = bpk.copy()
        bpkc[:, o_ea:o_ea + T] = 0.0
        eas = np.zeros(L, np.float32)
        eas[slot] = ak
        bpkc[:, o_ea:o_ea + T] = eas.reshape(T, P).T
        gsl = np.full(WP, -1.0, np.float32)
        gsl[:nloc] = (batch[n0:n1] - c["g0"]).astype(np.float32)
        bpkc[:, o_gsl:o_gsl + Wmax] = gsl.reshape(Wmax, P).T

        fpkc = fpk.copy()
        ng = c["g1"] - c["g0"]
        icnt = np.ones(GSLOT, np.float32)
        icnt[:ng] = 1.0 / np.maximum(cnt_g[c["g0"]:c["g1"]], 1.0)
        fpkc[:, 0:GSLOT] = np.tile(icnt.reshape(1, GSLOT), (D, 1))

        m = dict(
            t_xsw=xsw.astype(bf16),
            t_i16=i16.reshape(3, L // 16, 16).transpose(2, 0, 1)
                     .reshape(16, 3 * (L // 16)).copy(),
            t_bfp=bpkc.astype(bf16),
            t_fpk=fpkc,
        )
        in_maps.append(m)

    meta = dict(N=N, IN_C=IN_C, CHX=CHX, G=G, Wmax=Wmax, WP=WP,
                CPWlo=CPWlo, CPWhi=CPWhi, CPW=CPW, T=T, L=L, gb=gb,
                offs=dict(ea=o_ea, att=o_att, wec=o_wec, ioc=o_ioc,
                          i16=o_i16, iop=o_iop, gsl=o_gsl, w12=o_w12,
                          bcols=BCOLS))
    return meta, in_maps


# --------------------------------------------------------------------------
# bass program (single launch, collectives inside)
# --------------------------------------------------------------------------

def _build(meta, leaky_mode="prelu", debug=False):
    import concourse.bacc as bacc
    import concourse.mybir as mybir
    import concourse.tile as tile

    F32 = mybir.dt.float32
    BF = mybir.dt.bfloat16
    I16 = mybir.dt.int16
    AL = mybir.AluOpType
    AF = mybir.ActivationFunctionType
    AX = mybir.AxisListType

    N = meta["N"]
    CHX = meta["CHX"]
    Wmax, WP = meta["Wmax"], meta["WP"]
    CPWlo, CPWhi, CPW = meta["CPWlo"], meta["CPWhi"], meta["CPW"]
    T, L = meta["T"], meta["L"]
    SPLIT = (NC // 2) * WP
    O = meta["offs"]

    nc = bacc.Bacc(None, target_bir_lowering=False, num_devices=NC, debug=debug)

    t_xsw = nc.dram_tensor("t_xsw", [CHX, WP + 3 * D], BF, kind="ExternalInput")
    t_i16 = nc.dram_tensor("t_i16", [16, 3 * (L // 16)], I16,
                           kind="ExternalInput")
    t_bfp = nc.dram_tensor("t_bfp", [P, O["bcols"]], BF, kind="ExternalInput")
    t_fpk = nc.dram_tensor("t_fpk", [D, GSLOT + 4], F32, kind="ExternalInput")

    o_out = nc.dram_tensor("o_out", [2, GSLOT], F32, kind="ExternalOutput")

    xl_sh = nc.dram_tensor("xl_sh", [WP, D], F32)
    xl_full = nc.dram_tensor("xl_full", [NC * WP, D], F32)
    xr_tab = nc.dram_tensor("xr_tab", [WP, D], F32)
    oh_tab = nc.dram_tensor("oh_tab", [2 * P, P], BF)
    bn_in = nc.dram_tensor("bn_in", [P, 1], F32)
    bn_out = nc.dram_tensor("bn_out", [P, 1], F32)

    PL = _pieces(CPWlo)
    PH = _pieces(CPWhi)

    with tile.TileContext(nc) as tc:
        with tc.tile_pool(name="cst", bufs=1) as cst, \
             tc.tile_pool(name="pa", bufs=2, space="PSUM") as pa_pool, \
             tc.tile_pool(name="win", bufs=2, space="PSUM") as win_pool, \
             tc.tile_pool(name="acc", bufs=1, space="PSUM") as acc_pool, \
             tc.tile_pool(name="rsm", bufs=1, space="PSUM") as rsm_pool, \
             tc.tile_pool(name="gat", bufs=3) as gatp, \
             tc.tile_pool(name="wrk", bufs=3) as wrk:

            xsw_t = cst.tile([CHX, WP + 3 * D], BF, tag="xsw")
            nc.sync.dma_start(xsw_t[:], t_xsw[:])
            bfp_t = cst.tile([P, O["bcols"]], BF, tag="bfp")
            nc.sync.dma_start(bfp_t[:], t_bfp[:])
            fpk_t = cst.tile([D, GSLOT + 4], F32, tag="fpk")
            nc.sync.dma_start(fpk_t[:], t_fpk[:])
            # gather index streams: replicate [16, .] across the 8 gpsimd
            # stripes on device
            srct = cst.tile([P, L // 16], I16, tag="srct")
            dstt = cst.tile([P, L // 16], I16, tag="dstt")
            reltt = cst.tile([P, L // 16], I16, tag="reltt")
            for r in range(8):
                nc.sync.dma_start(srct[16 * r:16 * (r + 1), :],
                                  t_i16[:, 0:L // 16])
                nc.sync.dma_start(dstt[16 * r:16 * (r + 1), :],
                                  t_i16[:, L // 16:2 * (L // 16)])
                nc.sync.dma_start(reltt[16 * r:16 * (r + 1), :],
                                  t_i16[:, 2 * (L // 16):3 * (L // 16)])

            ea_v = bfp_t[:, O["ea"]:O["ea"] + T]
            attc_v = bfp_t[:, O["att"]:O["att"] + PIECE * D]
            wec_v = bfp_t[:, O["wec"]:O["wec"] + PIECE * D]
            iotac_v = bfp_t[:, O["ioc"]:O["ioc"] + P]
            iota16_v = bfp_t[:, O["i16"]:O["i16"] + GSLOT]
            w1_v = bfp_t[0:D + 1, O["w12"]:O["w12"] + D]
            w2_v = bfp_t[0:D + 1, O["w12"] + D:O["w12"] + D + 2]
            icnt_v = fpk_t[:, 0:GSLOT]
            misc_v = fpk_t[:, GSLOT:GSLOT + 4]

            iopf = cst.tile([P, 1], F32, tag="iopf")
            nc.vector.tensor_copy(iopf[:], bfp_t[:, O["iop"]:O["iop"] + 1])
            gslf_t = cst.tile([P, Wmax], F32, tag="gslf")
            nc.vector.tensor_copy(gslf_t[:], bfp_t[:, O["gsl"]:O["gsl"] + Wmax])
            gm_all = cst.tile([P, Wmax, GSLOT], BF, tag="gmall")

            # one-hot gather table: identity rows then a zero row block
            idt = wrk.tile([P, P], BF, tag="idt")
            nc.vector.tensor_scalar(idt[:], iotac_v, iopf[:], None, AL.is_equal)
            nc.gpsimd.dma_start(oh_tab[0:P, :], idt[:])
            zt = wrk.tile([P, P], BF, tag="zt")
            nc.vector.memset(zt[:], 0.0)
            nc.gpsimd.dma_start(oh_tab[P:2 * P, :], zt[:])

            ps_rsum = rsm_pool.tile([D, GSLOT], F32, tag="rsum")
            ps_stats = acc_pool.tile([P, GSLOT], F32, tag="stats")

            # ---------------- phase A: projection tables -----------------
            for w in range(Wmax):
                ps_a = pa_pool.tile([P, 3 * D], F32, tag="pa")
                nc.tensor.matmul(ps_a[:], xsw_t[:, w * P:(w + 1) * P],
                                 xsw_t[:, WP:WP + 3 * D], start=True, stop=True,
                                 skip_group_check=True)
                sxl = wrk.tile([P, D], F32, tag="sxl")
                nc.scalar.activation(sxl[:], ps_a[:, 0:D], AF.Copy)
                nc.gpsimd.dma_start(xl_sh[w * P:(w + 1) * P, :], sxl[:])
                sxr = wrk.tile([P, D], F32, tag="sxr")
                nc.scalar.activation(sxr[:], ps_a[:, D:2 * D], AF.Copy)
                nc.gpsimd.dma_start(xr_tab[w * P:(w + 1) * P, :], sxr[:])
                sxe = wrk.tile([P, D], BF, tag="sxe")
                nc.scalar.activation(sxe[:], ps_a[:, 2 * D:3 * D], AF.Copy)
                nc.vector.tensor_scalar(gm_all[:, w, :], iota16_v,
                                        gslf_t[:, w:w + 1], None, AL.is_equal)
                nc.tensor.matmul(ps_rsum[:], sxe[:], gm_all[:, w, :],
                                 start=(w == 0), stop=(w == Wmax - 1),
                                 skip_group_check=True)

            nc.gpsimd.collective_compute(
                "AllGather", AL.bypass,
                replica_groups=[list(range(NC))],
                ins=[xl_sh[:]], outs=[xl_full[:]],
            )

            # ---------------- edge loop --------------------------------
            for w in range(Wmax):
                win_ps = win_pool.tile([P, D + HEADS], F32, tag="win")
                cw = 0  # chunk index within window
                for run_off, run_pieces, lo in ((0, PL, True),
                                                (CPWlo, PH, False)):
                    po = 0
                    for m in run_pieces:
                        c0 = w * CPW + run_off + po   # global chunk
                        so = c0 * P                   # global slot
                        gx = gatp.tile([P, m, D], F32, tag=f"gx{m}")
                        src_tab = xl_full[0:SPLIT, :] if lo \
                            else xl_full[SPLIT:2 * SPLIT, :]
                        nc.gpsimd.dma_gather(
                            out_ap=gx[:], in_ap=src_tab,
                            idxs_ap=srct[:, so // 16:(so + m * P) // 16],
                            num_idxs=m * P, num_idxs_reg=m * P, elem_size=D)
                        gr = gatp.tile([P, m, D], F32, tag=f"gr{m}")
                        nc.gpsimd.dma_gather(
                            out_ap=gr[:], in_ap=xr_tab[:],
                            idxs_ap=dstt[:, so // 16:(so + m * P) // 16],
                            num_idxs=m * P, num_idxs_reg=m * P, elem_size=D)
                        oh = gatp.tile([P, m, P], BF, tag=f"oh{m}")
                        nc.gpsimd.dma_gather(
                            out_ap=oh[:], in_ap=oh_tab[:],
                            idxs_ap=reltt[:, so // 16:(so + m * P) // 16],
                            num_idxs=m * P, num_idxs_reg=m * P, elem_size=P)

                        em = wrk.tile([P, m, D], F32, tag=f"em{m}")
                        nc.vector.tensor_tensor(
                            out=em[:],
                            in0=ea_v[:, c0:c0 + m].unsqueeze(2)
                                .to_broadcast([P, m, D]),
                            in1=wec_v[:, 0:m * D].rearrange(
                                "p (c f) -> p c f", c=m),
                            op=AL.mult)
                        sa = wrk.tile([P, m, D], F32, tag=f"sa{m}")
                        nc.vector.tensor_tensor(out=sa[:], in0=gx[:],
                                                in1=gr[:], op=AL.add)
                        nc.vector.tensor_tensor(out=sa[:], in0=sa[:],
                                                in1=em[:], op=AL.add)
                        sb_t = wrk.tile([P, m, D], BF, tag=f"t{m}")
                        if leaky_mode == "prelu":
                            nc.scalar.activation(sb_t[:], sa[:], AF.Prelu,
                                                 alpha=NEG_SLOPE)
                        else:
                            sb_r = wrk.tile([P, m, D], F32, tag=f"r{m}")
                            nc.scalar.activation(sb_r[:], sa[:], AF.Relu,
                                                 scale=-(1.0 - NEG_SLOPE))
                            nc.vector.tensor_tensor(out=sb_t[:], in0=sa[:],
                                                    in1=sb_r[:], op=AL.add)
                        sb_u = wrk.tile([P, m, D], BF, tag=f"u{m}")
                        nc.vector.tensor_tensor(
                            out=sb_u[:], in0=sb_t[:],
                            in1=attc_v[:, 0:m * D].rearrange(
                                "p (c f) -> p c f", c=m),
                            op=AL.mult)
                        sb_lg = wrk.tile([P, m, HEADS], F32, tag=f"lg{m}")
                        nc.vector.tensor_reduce(
                            out=sb_lg[:],
                            in_=sb_u[:].rearrange("p c (h k) -> p c h k",
                                                  k=OUT_C),
                            axis=AX.X, op=AL.add)
                        exf = wrk.tile([P, m, HEADS], F32, tag=f"ex{m}")
                        nc.scalar.activation(exf[:], sb_lg[:], AF.Exp)
                        exb = wrk.tile([P, m, D], F32, tag=f"exb{m}")
                        nc.scalar.activation(
                            exb[:].rearrange("p c (h k) -> p c h k", k=OUT_C),
                            exf[:].unsqueeze(3).to_broadcast(
                                [P, m, HEADS, OUT_C]),
                            AF.Copy)
                        msg = wrk.tile([P, m, D + HEADS], BF, tag=f"msg{m}")
                        nc.vector.tensor_tensor(out=msg[:, :, 0:D], in0=gx[:],
                                                in1=exb[:], op=AL.mult)
                        nc.scalar.activation(msg[:, :, D:D + HEADS], exf[:],
                                             AF.Copy)
                        for j in range(m):
                            nc.tensor.matmul(win_ps[:], oh[:, j, :],
                                             msg[:, j, :],
                                             start=(cw + j == 0),
                                             stop=(cw + j == CPW - 1),
                                             skip_group_check=True)
                        po += m
                        cw += m

                # window flush: softmax-normalize, accumulate BN/pool stats
                den = wrk.tile([P, HEADS], F32, tag="den")
                nc.vector.tensor_scalar(den[:], win_ps[:, D:D + HEADS],
                                        1e-20, None, AL.add)
                rd = wrk.tile([P, HEADS], F32, tag="rd")
                nc.vector.reciprocal(rd[:], den[:])
                hh2 = wrk.tile([P, 2 * D], BF, tag="hh2")
                nc.vector.tensor_tensor(
                    out=hh2[:, 0:D].rearrange("p (h k) -> p h k", k=OUT_C),
                    in0=win_ps[:, 0:D].rearrange("p (h k) -> p h k", k=OUT_C),
                    in1=rd[:].unsqueeze(2).to_broadcast([P, HEADS, OUT_C]),
                    op=AL.mult)
                nc.scalar.activation(hh2[:, D:2 * D], hh2[:, 0:D], AF.Square)
                nc.tensor.matmul(ps_stats[:], hh2[:], gm_all[:, w, :],
                                 start=(w == 0), stop=(w == Wmax - 1),
                                 skip_group_check=True)

            # ---------------- BN allreduce + tail ----------------------
            sl = wrk.tile([P, 1], F32, tag="sl")
            nc.vector.tensor_reduce(out=sl[:], in_=ps_stats[:],
                                    axis=AX.X, op=AL.add)
            nc.gpsimd.dma_start(bn_in[:], sl[:])
            nc.gpsimd.collective_compute(
                "AllReduce", AL.add,
                replica_groups=[list(range(NC))],
                ins=[bn_in[:]], outs=[bn_out[:]],
            )
            sh = wrk.tile([D, 2], F32, tag="sh")
            nc.gpsimd.dma_start(sh[:, 0:1], bn_out[0:D, :])
            nc.gpsimd.dma_start(sh[:, 1:2], bn_out[D:2 * D, :])

            mu = wrk.tile([D, 1], F32, tag="mu")
            nc.scalar.activation(mu[:], sh[:, 0:1], AF.Copy, scale=1.0 / N)
            e2 = wrk.tile([D, 1], F32, tag="e2")
            nc.scalar.activation(e2[:], sh[:, 1:2], AF.Copy, scale=1.0 / N)
            mu2 = wrk.tile([D, 1], F32, tag="mu2")
            nc.scalar.activation(mu2[:], mu[:], AF.Square)
            var = wrk.tile([D, 1], F32, tag="var")
            nc.vector.tensor_tensor(out=var[:], in0=e2[:], in1=mu2[:],
                                    op=AL.subtract)
            sd = wrk.tile([D, 1], F32, tag="sd")
            nc.scalar.activation(sd[:], var[:], AF.Sqrt, bias=misc_v[:, 2:3])
            rsd = wrk.tile([D, 1], F32, tag="rsd")
            nc.vector.reciprocal(rsd[:], sd[:])
            cA = wrk.tile([D, 1], F32, tag="cA")
            nc.vector.tensor_tensor(out=cA[:], in0=misc_v[:, 0:1], in1=rsd[:],
                                    op=AL.mult)
            tmp = wrk.tile([D, 1], F32, tag="tmp")
            nc.vector.tensor_tensor(out=tmp[:], in0=cA[:], in1=mu[:],
                                    op=AL.mult)
            cB = wrk.tile([D, 1], F32, tag="cB")
            nc.vector.tensor_tensor(out=cB[:], in0=misc_v[:, 1:2], in1=tmp[:],
                                    op=AL.subtract)

            ph = wrk.tile([D, GSLOT], F32, tag="ph")
            nc.vector.tensor_tensor(out=ph[:], in0=ps_stats[0:D, :],
                                    in1=icnt_v, op=AL.mult)
            pooled = wrk.tile([D, GSLOT], F32, tag="pooled")
            nc.vector.tensor_scalar(pooled[:], ph[:], cA[:], cB[:],
                                    AL.mult, AL.add)
            pr = wrk.tile([D, GSLOT], F32, tag="pr")
            nc.vector.tensor_tensor(out=pr[:], in0=ps_rsum[:], in1=icnt_v,
                                    op=AL.mult)
            zr = wrk.tile([D + 1, GSLOT], BF, tag="zr")
            nc.vector.memset(zr[D:D + 1, :], 1.0)
            nc.vector.tensor_tensor(out=zr[0:D, :], in0=pooled[:], in1=pr[:],
                                    op=AL.add)
            ps_z = win_pool.tile([D, GSLOT], F32, tag="win")
            nc.tensor.matmul(ps_z[:], w1_v, zr[:], start=True, stop=True,
                             skip_group_check=True)
            z2 = wrk.tile([D + 1, GSLOT], BF, tag="z2")
            nc.vector.memset(z2[D:D + 1, :], 1.0)
            nc.scalar.activation(z2[0:D, :], ps_z[:], AF.Relu)
            ps_o = win_pool.tile([2, GSLOT], F32, tag="win")
            nc.tensor.matmul(ps_o[:], w2_v, z2[:], start=True, stop=True,
                             skip_group_check=True)
            so = wrk.tile([2, GSLOT], F32, tag="so")
            nc.scalar.activation(so[:], ps_o[:], AF.Copy)
            nc.sync.dma_start(o_out[:], so[:])

    nc.compile()
    return nc


# --------------------------------------------------------------------------
# entry point
# --------------------------------------------------------------------------

def _run_sim(nc, in_maps):
    from concourse.bass_interp import MultiCoreSim
    ms = MultiCoreSim(nc, num_cores=NC, num_workers=NC,
                      require_finite=False, require_nnan=False)
    for k in range(NC):
        for name, arr in in_maps[k].items():
            ms.cores[k].tensor(name)[:] = arr
    ms.simulate()
    return [{"o_out": np.array(ms.cores[k].tensor("o_out"))} for k in range(NC)]


def kernel(**inputs):
    meta, in_maps = _prep(inputs)
    key = ("v3", meta["IN_C"], meta["Wmax"], meta["CPWlo"], meta["CPWhi"],
           meta["N"], _LEAKY_MODE)
    if key not in _prog_cache:
        _prog_cache[key] = _build(meta, leaky_mode=_LEAKY_MODE,
                                  debug=(_RUN_MODE == "sim"))
    nc = _prog_cache[key]

    global LAST_EXEC_NS
    if _RUN_MODE == "sim":
        res = _run_sim(nc, in_maps)
        LAST_EXEC_NS = [None]
    else:
        from concourse.bass_utils import run_bass_kernel_spmd
        import time as _time
        _t0 = _time.time()
        r = run_bass_kernel_spmd(nc, in_maps, list(range(NC)))
        _t1 = _time.time()
        res = r.results
        LAST_EXEC_NS = [getattr(r, "exec_time_ns", None)
                        or int((_t1 - _t0) * 1e9)]

    G = meta["G"]
    gb = meta["gb"]
    out = np.zeros((G, 2), np.float32)
    for g in range(G):
        k = int(np.searchsorted(gb, g, side="right")) - 1
        slot = g - int(gb[k])
        out[g] = res[k]["o_out"][:, slot]
    return out


_LEAKY_MODE = "prelu"
_RUN_MODE = "hw"
LAST_EXEC_NS = None
